# revision 34
# baseline (speedup 1.0000x reference)
"""GAT-style graph attention kernel for Trainium2 (Bass/Tile), 8-core SPMD.

Per graph b (one NeuronCore each, B=8):
    X  = H[b] @ W                      [N, U]
    s  = X @ a_1   (per-query logit)   [N, 1]
    n  = X @ a_2   (per-key logit)     [N, 1]
    E  = leaky_relu(s_i + n_j, 0.2)    [N, N]
    P  = exp(E) * A[b]                 (== exp(E + NEG*(1-A)), A in {0,1})
    out= relu((P @ X) / rowsum(P))     [N, U]

v8 design (measured on HW, 8 cores concurrent):
  Transport: ONE DMA queue with a deep buffer pool sustains ~375 GB/s
  in isolation while two concurrent queues interleave at packet
  granularity and drop to ~305.  The whole A stream rides the gpsimd
  (SWDGE) queue as f32->f16-cast 2MiB singles with an 11-buffer pool;
  W/a1/H head the same queue; a2 rides sync; outputs accumulate in
  SBUF and flush in two DMAs (mid-stream + end).
  Compute: exp(leaky(t)) = max(exp(t), exp(0.2 t)), t = s_i + n_j, and
  both branches factor rank-1: exp(t) = exp(s_i)exp(n_j).  Per-iter op
  menu (measured): ACT pass 3.9us (any activation, 1 elem/lane/cyc),
  DVE tensor_scalar 1.3us (4x), tensor_tensor 2.3us (2x), PSUM copy
  0.75us/2k (4x).  scalar_tensor_tensor is 1x (4.5us) - avoid.  GpSimd
  tensor ops are ~5us AND break DVE fast modes (SBUF port contention) -
  never use them.  The balanced mix: 22 ACT-heavy iters (Prelu+Exp,
  DVE idle) + 10 pure-DVE iters (x1 = es*en, x2 = w*z via
  tensor_scalar, then max) + mask (one full in-place tensor_mul) +
  PSUM->SBUF copies on DVE.  This is the LP optimum of the op menu:
  ACT ~= DVE ~= 5.4us/iter ideal, ~6.2 with the chip's utilization
  throttle (activity-based duty cycling, ~0.63 avg limit), just above
  the ~6.0us/iter throttled DMA pace.
"""

import numpy as np
from contextlib import ExitStack

import concourse.bass as bass
import concourse.bacc as bacc
import concourse.mybir as mybir
import concourse.tile as tile
from concourse.masks import make_identity

F32 = mybir.dt.float32
F16 = mybir.dt.float16

N_NODES = 4096
N_FEAT = 128
N_UNITS = 64
N_CORES = 8
LEAKY_SLOPE = 0.2

A_BUFS = 10      # f16 A-tile pool depth (deep => SWDGE singles at line rate)
# P5 iterations produce p purely on DVE via rank-1 factors:
#   x1 = exp(s_i)*exp(n_j), x2 = exp(0.2 s_i)*exp(0.2 n_j), p = max(x1,x2)
# (two tensor_scalar + one max).  All other iterations are ACT-heavy P1
# (Prelu then Exp).  22:10 balances ACT ~5.3us vs DVE ~5.5us per iter.
P5_ITERS = (0, 3, 6, 9, 12, 15, 18, 21, 24, 27, 30)
N_PRE = 4        # A loads issued before anything else on the gpsimd queue
FLUSH_SPLIT = True  # flush first half of outputs mid-stream


def build_nc(n_nodes=N_NODES):
    P = 128  # partitions
    U = N_UNITS
    F = N_FEAT
    n_t = n_nodes // P          # node tiles (32 full size)
    assert n_nodes % P == 0

    nc = bacc.Bacc(None)
    H_d = nc.declare_dram_parameter("H", [n_nodes, F], F32, isOutput=False)
    A_d = nc.declare_dram_parameter("A", [n_nodes, n_nodes], F32, isOutput=False)
    W_d = nc.declare_dram_parameter("W", [F, U], F32, isOutput=False)
    a1_d = nc.declare_dram_parameter("a_1", [U, 1], F32, isOutput=False)
    a2_d = nc.declare_dram_parameter("a_2", [U, 1], F32, isOutput=False)
    out_d = nc.declare_dram_parameter("out", [n_nodes, U], F32, isOutput=True)

    M = mybir.AluOpType
    AF = mybir.ActivationFunctionType

    with tile.TileContext(nc) as tc, ExitStack() as ctx:
        const = ctx.enter_context(tc.tile_pool(name="const", bufs=1))
        persist = ctx.enter_context(tc.tile_pool(name="persist", bufs=1))
        # A stream: deep f16 pool, one queue, strictly sequential issue.
        apool = ctx.enter_context(tc.tile_pool(name="apool", bufs=min(A_BUFS, n_t)))

        a_tiles = {}
        next_a = [0]

        def load_a():
            it = next_a[0]
            if it >= n_t:
                return
            next_a[0] = it + 1
            t = apool.tile([P, n_nodes], F16, tag="a16")
            nc.gpsimd.dma_start(t[:], A_d[it * P:(it + 1) * P, :])
            a_tiles[it] = t

        # Small weights + H ride the gpsimd queue (f32->f16 cast) AHEAD of
        # the A singles; a2 (kept f32) rides sync.
        W_sb = const.tile([F, U], F16)
        nc.gpsimd.dma_start(W_sb[:], W_d[:])
        a1_sb = const.tile([U, 1], F16)
        nc.gpsimd.dma_start(a1_sb[:], a1_d[:])
        a2_sb = const.tile([U, 1], F32)
        nc.sync.dma_start(a2_sb[:], a2_d[:])

        ident16 = const.tile([P, P], F16)

        # a2 broadcast along free dim: a2b[u, c] = a2[u]
        a2b = const.tile([U, P], F16)
        nc.vector.memset(a2b[:], 1.0)
        nc.vector.tensor_scalar_mul(a2b[:], a2b[:], a2_sb[:, 0:1])
        WT_sb = const.tile([U, F], F16)       # W^T
        wa1_sb = const.tile([F, 1], F16)      # W @ a_1
        wa2b_sb = const.tile([F, P], F16)     # (W @ a_2) bcast along free

        # persistent per-graph tensors
        n_bcast = persist.tile([P, n_nodes], F16)     # n[j] bcast over partitions
        z_b = persist.tile([P, n_nodes], F16)         # exp(0.2 n[j]) bcast
        en_b = persist.tile([P, n_nodes], F16)        # exp(n[j]) bcast
        Xp_sb = persist.tile([P, n_t * (U + 1)], F16)  # X' tiles [X_t | 1]
        s_sb = persist.tile([P, n_t], F32)            # s column per query tile
        w_sb = persist.tile([P, n_t], F32)            # exp(0.2 s)
        es_sb = persist.tile([P, n_t], F32)           # exp(s)
        dinv_sb = persist.tile([P, n_t], F32)
        outsbuf = persist.tile([P, n_t * U], F32)     # all outputs, flushed late
        nc.vector.memset(Xp_sb[:], 1.0)

        HCH = max(1, n_t // 4)

        # ---------------- prep: X, X^T, s, z_b, n_bcast ----------------
        with tc.tile_pool(name="hpool", bufs=1) as hpool, \
             tc.tile_pool(name="prep", bufs=6) as prep, \
             tc.tile_pool(name="prep_ps", bufs=2, space="PSUM") as prep_ps, \
             tc.tile_pool(name="prep_ps1", bufs=2, space="PSUM") as prep_ps1:

            h_chunks = {}
            for c in range(0, n_t, HCH):
                hc = hpool.tile([P, HCH * F], F16, tag=f"h16_{c}")
                nc.gpsimd.dma_start(
                    hc[:].rearrange("p (t f) -> p t f", f=F),
                    H_d[c * P:(c + HCH) * P, :].rearrange(
                        "(t p) f -> p t f", p=P))
                h_chunks[c] = hc

            # identity on gpsimd compute, then the first A emissions
            make_identity(nc, ident16[:])
            for _ in range(min(N_PRE, n_t)):
                load_a()

            # Fold the tiny head matmuls: W^T, then wa1 = W@a_1 and
            # wa2b[f, c] = (W@a_2)[f] so s, n_bcast and the X tiles all
            # come straight from h^T (no X^T tensor at all).
            wt_ps = prep_ps.tile([U, F], F16, tag="hT_ps")
            nc.tensor.transpose(wt_ps[:], W_sb[:], ident16[:])
            nc.scalar.copy(WT_sb[:], wt_ps[:])
            wa_ps = prep_ps.tile([F, P + 1], F32, tag="nb_ps")
            nc.tensor.matmul(wa_ps[:, 0:1], WT_sb[:], a1_sb[:],
                             start=True, stop=True)
            nc.tensor.matmul(wa_ps[:, 1:P + 1], WT_sb[:], a2b[:],
                             start=True, stop=True)
            nc.scalar.copy(wa1_sb[:], wa_ps[:, 0:1])
            nc.vector.tensor_copy(wa2b_sb[:], wa_ps[:, 1:P + 1])

            QB = 4 if n_t % 4 == 0 else 2
            for t2 in range(0, n_t, QB):
                hT_ps = prep_ps.tile([P, QB * P], F16, tag="hT_ps")
                for k in range(QB):
                    t = t2 + k
                    hc = h_chunks[(t // HCH) * HCH]
                    nc.tensor.transpose(hT_ps[:, k * P:k * P + F],
                                        hc[:, (t % HCH) * F:(t % HCH + 1) * F],
                                        ident16[:])
                hT_sb = prep.tile([F, QB * P], F16)
                nc.scalar.copy(hT_sb[:], hT_ps[:F, 0:QB * P])
                # s[p, t] = (H_t @ (W a1))[p]
                s_q = prep_ps1.tile([P, QB], F32, tag="s_q")
                for k in range(QB):
                    nc.tensor.matmul(s_q[:, k:k + 1],
                                     hT_sb[:, k * P:(k + 1) * P],
                                     wa1_sb[:], start=True, stop=True)
                nc.vector.tensor_copy(s_sb[:, t2:t2 + QB], s_q[:])
                # n_bcast[p, slice] = n[slice] broadcast over partitions
                nb_ps = prep_ps.tile([P, QB * P], F32, tag="nb_ps")
                nc.tensor.matmul(nb_ps[:], wa2b_sb[:], hT_sb[:],
                                 start=True, stop=True)
                nc.vector.tensor_copy(n_bcast[:, t2 * P:(t2 + QB) * P],
                                      nb_ps[:])
                # X tiles straight from h^T: X_t = H_t @ W
                for k in range(QB):
                    t = t2 + k
                    x_ps = prep_ps.tile([P, U], F32, tag="xq")
                    nc.tensor.matmul(x_ps[:], hT_sb[:, k * P:(k + 1) * P],
                                     W_sb[:], start=True, stop=True)
                    nc.vector.tensor_copy(
                        Xp_sb[:, t * (U + 1):t * (U + 1) + U], x_ps[:])
                # z_b = exp(0.2 n), en_b = exp(n) straight from PSUM on ACT
                nc.scalar.activation(z_b[:, t2 * P:(t2 + QB) * P], nb_ps[:],
                                     AF.Exp, scale=LEAKY_SLOPE)
                nc.scalar.activation(en_b[:, t2 * P:(t2 + QB) * P], nb_ps[:],
                                     AF.Exp)
            # w = exp(0.2 s), es = exp(s) per-partition scalars
            nc.scalar.activation(w_sb[:], s_sb[:], AF.Exp, scale=LEAKY_SLOPE)
            nc.scalar.activation(es_sb[:], s_sb[:], AF.Exp)

        # ---------------- main loop over query tiles ----------------
        p5set = set(i for i in P5_ITERS if i < n_t)
        GROUP = 16                     # transposes per PSUM tile (2 banks)
        n_groups = (n_t + GROUP - 1) // GROUP
        LOOK = 2                       # produce lookahead (iters)

        with tc.tile_pool(name="x1pool", bufs=2) as x1pool, \
             tc.tile_pool(name="x2pool", bufs=1) as x2pool, \
             tc.tile_pool(name="ppool", bufs=LOOK + 2) as ppool, \
             tc.tile_pool(name="ptpool", bufs=4) as ptpool, \
             tc.tile_pool(name="psT", bufs=3, space="PSUM") as psT, \
             tc.tile_pool(name="psAcc", bufs=2, space="PSUM") as psAcc:

            p_tiles = {}
            acc_tiles = {}

            def produce(it):
                load_a()               # keep the gpsimd queue fed, in order
                s_bias = s_sb[:, it:it + 1]
                p_t = ppool.tile([P, n_nodes], F16, tag="p")
                if it in p5set:
                    # pure-DVE: x1 = es_i*en_j, x2 = w_i*z_j, p = max
                    x1 = x1pool.tile([P, n_nodes], F16, tag="x1")
                    nc.vector.tensor_scalar_mul(x1[:], en_b[:],
                                                es_sb[:, it:it + 1])
                    x2 = x2pool.tile([P, n_nodes], F16, tag="x2")
                    nc.vector.tensor_scalar_mul(x2[:], z_b[:], w_sb[:, it:it + 1])
                    nc.vector.tensor_max(p_t[:], x1[:], x2[:])
                else:
                    # ACT-heavy: Prelu then Exp (both ScalarE, no DVE)
                    el = x1pool.tile([P, n_nodes], F16, tag="el")
                    nc.scalar.activation(el[:], n_bcast[:], AF.Prelu,
                                         bias=s_bias, scale=1.0,
                                         alpha=LEAKY_SLOPE)
                    nc.scalar.activation(p_t[:], el[:], AF.Exp)
                p_tiles[it] = p_t

            def consume(it):
                a_t = a_tiles.pop(it)
                p_t = p_tiles.pop(it)
                fine = it >= n_t - 2   # tail iterations: 8-block pipelining
                half = n_nodes // 2
                if not fine:
                    # mask in place on DVE, one full pass (fewer drains;
                    # never GpSimd: its tensor ops contend with DVE 2-port
                    # mode and slow everything down)
                    nc.vector.tensor_mul(p_t[:], p_t[:], a_t[:])

                # transpose P_m 128x128 blocks -> PSUM, copy groups to SBUF
                acc_ps = psAcc.tile([P, U + 1], F32, tag="acc_ps")
                for g in range(n_groups):
                    k_n = min(GROUP, n_t - g * GROUP)
                    pt_ps = psT.tile([P, GROUP * P], F16, tag="pt_ps")
                    for half_g in range(2 if fine else 1):
                        if fine:
                            lo = g * GROUP * P + half_g * (GROUP // 2) * P
                            hi = lo + (GROUP // 2) * P
                            nc.vector.tensor_mul(p_t[:, lo:hi], p_t[:, lo:hi],
                                                 a_t[:, lo:hi])
                            ks = range(half_g * (GROUP // 2),
                                       min(k_n, (half_g + 1) * (GROUP // 2)))
                        else:
                            ks = range(k_n)
                        for k in ks:
                            jt = g * GROUP + k
                            nc.tensor.transpose(pt_ps[:, k * P:(k + 1) * P],
                                                p_t[:, jt * P:(jt + 1) * P],
                                                ident16[:])
                    pt_sb = ptpool.tile([P, GROUP * P], F16, tag="pt_sb")
                    w_n = k_n * P
                    if fine:
                        # split the copy across both engines in the tail
                        nc.scalar.copy(pt_sb[:, 0:w_n // 2], pt_ps[:, 0:w_n // 2])
                        nc.vector.tensor_copy(pt_sb[:, w_n // 2:w_n],
                                              pt_ps[:, w_n // 2:w_n])
                    else:
                        nc.vector.tensor_copy(pt_sb[:, 0:w_n], pt_ps[:, 0:w_n])
                    # H_cap accumulation for this group's j tiles
                    for k in range(k_n):
                        jt = g * GROUP + k
                        nc.tensor.matmul(
                            acc_ps[:], pt_sb[:, k * P:(k + 1) * P],
                            Xp_sb[:, jt * (U + 1):(jt + 1) * (U + 1)],
                            start=(jt == 0), stop=(jt == n_t - 1))

                nc.vector.reciprocal(dinv_sb[:, it:it + 1], acc_ps[:, U:U + 1])
                acc_tiles[it] = acc_ps

            def emit_out(it):
                # out = relu(H_cap[:, :U] / H_cap[:, U]) -- relu+scale on ACT,
                # into the SBUF output buffer (flushed by DMA at the end).
                acc_ps = acc_tiles.pop(it)
                nc.scalar.activation(outsbuf[:, it * U:(it + 1) * U],
                                     acc_ps[:, 0:U], AF.Relu,
                                     scale=dinv_sb[:, it:it + 1])

            for it in range(n_t + LOOK + 1):
                if it < n_t:
                    produce(it)
                if LOOK <= it < n_t + LOOK:
                    ct = it - LOOK
                    consume(ct)
                    if ct >= n_t - 2:
                        emit_out(ct)
                if LOOK < it < n_t + LOOK - 1:
                    emit_out(it - LOOK - 1)
                if FLUSH_SPLIT and it == n_t * 5 // 8:
                    hn = n_t // 2
                    nc.sync.dma_start(
                        out_d[0:hn * P, :].rearrange("(t p) u -> p t u", p=P),
                        outsbuf[:, 0:hn * U].rearrange("p (t u) -> p t u", u=U))

            # final output flush on the idle sync ring
            lo = (n_t // 2) if FLUSH_SPLIT else 0
            nc.sync.dma_start(
                out_d[lo * P:n_t * P, :].rearrange("(t p) u -> p t u", p=P),
                outsbuf[:, lo * U:n_t * U].rearrange("p (t u) -> p t u", u=U))

    nc.compile()
    return nc


_NC_CACHE = {}


def _get_nc(n_nodes=N_NODES):
    if n_nodes not in _NC_CACHE:
        _NC_CACHE[n_nodes] = build_nc(n_nodes)
    return _NC_CACHE[n_nodes]


def kernel(H, A, W, a_1, a_2):
    """Full inputs in, full output out. Shards batch across 8 NeuronCores."""
    import os
    # The axon trace path needs antenv.axon_hooks, which this image lacks;
    # make sure an inherited BASS_TRACE can't route us there.
    os.environ["BASS_NEVER_TRACE"] = "1"
    from concourse.bass_utils import run_bass_kernel_spmd

    B = H.shape[0]
    assert B == N_CORES
    nc = _get_nc(H.shape[1])
    in_maps = [
        {
            "H": np.ascontiguousarray(H[b], dtype=np.float32),
            "A": np.ascontiguousarray(A[b], dtype=np.float32),
            "W": np.ascontiguousarray(W, dtype=np.float32),
            "a_1": np.ascontiguousarray(a_1, dtype=np.float32),
            "a_2": np.ascontiguousarray(a_2, dtype=np.float32),
        }
        for b in range(B)
    ]
    res = run_bass_kernel_spmd(nc, in_maps, core_ids=list(range(N_CORES)))
    out = np.stack([res.results[b]["out"] for b in range(B)]).astype(np.float32)
    return out


# revision 36
# speedup vs baseline: 1.0225x; 1.0225x over previous
"""GAT-style graph attention kernel for Trainium2 (Bass/Tile), 8-core SPMD.

Per graph b (one NeuronCore each, B=8):
    X  = H[b] @ W                      [N, U]
    s  = X @ a_1   (per-query logit)   [N, 1]
    n  = X @ a_2   (per-key logit)     [N, 1]
    E  = leaky_relu(s_i + n_j, 0.2)    [N, N]
    P  = exp(E) * A[b]                 (== exp(E + NEG*(1-A)), A in {0,1})
    out= relu((P @ X) / rowsum(P))     [N, U]

v8 design (measured on HW, 8 cores concurrent):
  Transport: ONE DMA queue with a deep buffer pool sustains ~375 GB/s
  in isolation while two concurrent queues interleave at packet
  granularity and drop to ~305.  The whole A stream rides the gpsimd
  (SWDGE) queue as f32->f16-cast 2MiB singles with an 11-buffer pool;
  W/a1/H head the same queue; a2 rides sync; outputs accumulate in
  SBUF and flush in two DMAs (mid-stream + end).
  Compute: exp(leaky(t)) = max(exp(t), exp(0.2 t)), t = s_i + n_j, and
  both branches factor rank-1: exp(t) = exp(s_i)exp(n_j).  Per-iter op
  menu (measured): ACT pass 3.9us (any activation, 1 elem/lane/cyc),
  DVE tensor_scalar 1.3us (4x), tensor_tensor 2.3us (2x), PSUM copy
  0.75us/2k (4x).  scalar_tensor_tensor is 1x (4.5us) - avoid.  GpSimd
  tensor ops are ~5us AND break DVE fast modes (SBUF port contention) -
  never use them.  The balanced mix: 22 ACT-heavy iters (Prelu+Exp,
  DVE idle) + 10 pure-DVE iters (x1 = es*en, x2 = w*z via
  tensor_scalar, then max) + mask (one full in-place tensor_mul) +
  PSUM->SBUF copies on DVE.  This is the LP optimum of the op menu:
  ACT ~= DVE ~= 5.4us/iter ideal, ~6.2 with the chip's utilization
  throttle (activity-based duty cycling, ~0.63 avg limit), just above
  the ~6.0us/iter throttled DMA pace.
"""

import numpy as np
from contextlib import ExitStack

import concourse.bass as bass
import concourse.bacc as bacc
import concourse.mybir as mybir
import concourse.tile as tile
from concourse.masks import make_identity

F32 = mybir.dt.float32
F16 = mybir.dt.float16

N_NODES = 4096
N_FEAT = 128
N_UNITS = 64
N_CORES = 8
LEAKY_SLOPE = 0.2

A_BUFS = 10      # f16 A-tile pool depth (deep => SWDGE singles at line rate)
# P5 iterations produce p purely on DVE via rank-1 factors:
#   x1 = exp(s_i)*exp(n_j), x2 = exp(0.2 s_i)*exp(0.2 n_j), p = max(x1,x2)
# (two tensor_scalar + one max).  All other iterations are ACT-heavy P1
# (Prelu then Exp).  22:10 balances ACT ~5.3us vs DVE ~5.5us per iter.
P5_ITERS = (0, 3, 6, 10, 13, 16, 20, 23, 26, 29)
N_PRE = 4        # A loads issued before anything else on the gpsimd queue
FLUSH_SPLIT = True  # flush first half of outputs mid-stream


def build_nc(n_nodes=N_NODES):
    P = 128  # partitions
    U = N_UNITS
    F = N_FEAT
    n_t = n_nodes // P          # node tiles (32 full size)
    assert n_nodes % P == 0

    nc = bacc.Bacc(None)
    H_d = nc.declare_dram_parameter("H", [n_nodes, F], F32, isOutput=False)
    A_d = nc.declare_dram_parameter("A", [n_nodes, n_nodes], F32, isOutput=False)
    W_d = nc.declare_dram_parameter("W", [F, U], F32, isOutput=False)
    a1_d = nc.declare_dram_parameter("a_1", [U, 1], F32, isOutput=False)
    a2_d = nc.declare_dram_parameter("a_2", [U, 1], F32, isOutput=False)
    out_d = nc.declare_dram_parameter("out", [n_nodes, U], F32, isOutput=True)

    M = mybir.AluOpType
    AF = mybir.ActivationFunctionType

    with tile.TileContext(nc) as tc, ExitStack() as ctx:
        const = ctx.enter_context(tc.tile_pool(name="const", bufs=1))
        persist = ctx.enter_context(tc.tile_pool(name="persist", bufs=1))
        # A stream: deep f16 pool, one queue, strictly sequential issue.
        apool = ctx.enter_context(tc.tile_pool(name="apool", bufs=min(A_BUFS, n_t)))

        a_tiles = {}
        next_a = [0]

        def load_a():
            it = next_a[0]
            if it >= n_t:
                return
            next_a[0] = it + 1
            t = apool.tile([P, n_nodes], F16, tag="a16")
            nc.gpsimd.dma_start(t[:], A_d[it * P:(it + 1) * P, :])
            a_tiles[it] = t

        # Small weights + H ride the gpsimd queue (f32->f16 cast) AHEAD of
        # the A singles; a2 (kept f32) rides sync.
        W_sb = const.tile([F, U], F16)
        nc.gpsimd.dma_start(W_sb[:], W_d[:])
        a1_sb = const.tile([U, 1], F16)
        nc.gpsimd.dma_start(a1_sb[:], a1_d[:])
        a2_sb = const.tile([U, 1], F32)
        nc.sync.dma_start(a2_sb[:], a2_d[:])

        ident16 = const.tile([P, P], F16)

        # a2 broadcast along free dim: a2b[u, c] = a2[u]
        a2b = const.tile([U, P], F16)
        nc.vector.memset(a2b[:], 1.0)
        nc.vector.tensor_scalar_mul(a2b[:], a2b[:], a2_sb[:, 0:1])
        WT_sb = const.tile([U, F], F16)       # W^T
        wa1_sb = const.tile([F, 1], F16)      # W @ a_1
        wa2b_sb = const.tile([F, P], F16)     # (W @ a_2) bcast along free

        # persistent per-graph tensors
        n_bcast = persist.tile([P, n_nodes], F16)     # n[j] bcast over partitions
        z_b = persist.tile([P, n_nodes], F16)         # exp(0.2 n[j]) bcast
        en_b = persist.tile([P, n_nodes], F16)        # exp(n[j]) bcast
        Xp_sb = persist.tile([P, n_t * (U + 1)], F16)  # X' tiles [X_t | 1]
        s_sb = persist.tile([P, n_t], F32)            # s column per query tile
        w_sb = persist.tile([P, n_t], F32)            # exp(0.2 s)
        es_sb = persist.tile([P, n_t], F32)           # exp(s)
        dinv_sb = persist.tile([P, n_t], F32)
        rs_sb = persist.tile([P, 2], F32)             # rowsum staging (recip pairs)
        outsbuf = persist.tile([P, n_t * U], F32)     # all outputs, flushed late
        nc.vector.memset(Xp_sb[:], 1.0)

        HCH = max(1, n_t // 4)

        # ---------------- prep: X, X^T, s, z_b, n_bcast ----------------
        with tc.tile_pool(name="hpool", bufs=1) as hpool, \
             tc.tile_pool(name="prep", bufs=6) as prep, \
             tc.tile_pool(name="prep_ps", bufs=2, space="PSUM") as prep_ps, \
             tc.tile_pool(name="prep_ps1", bufs=2, space="PSUM") as prep_ps1:

            h_chunks = {}
            for c in range(0, n_t, HCH):
                hc = hpool.tile([P, HCH * F], F16, tag=f"h16_{c}")
                nc.gpsimd.dma_start(
                    hc[:].rearrange("p (t f) -> p t f", f=F),
                    H_d[c * P:(c + HCH) * P, :].rearrange(
                        "(t p) f -> p t f", p=P))
                h_chunks[c] = hc

            # identity on gpsimd compute, then the first A emissions
            make_identity(nc, ident16[:])
            for _ in range(min(N_PRE, n_t)):
                load_a()

            # Fold the tiny head matmuls: W^T, then wa1 = W@a_1 and
            # wa2b[f, c] = (W@a_2)[f] so s, n_bcast and the X tiles all
            # come straight from h^T (no X^T tensor at all).
            wt_ps = prep_ps.tile([U, F], F16, tag="hT_ps")
            nc.tensor.transpose(wt_ps[:], W_sb[:], ident16[:])
            nc.scalar.copy(WT_sb[:], wt_ps[:])
            wa_ps = prep_ps.tile([F, P + 1], F32, tag="nb_ps")
            nc.tensor.matmul(wa_ps[:, 0:1], WT_sb[:], a1_sb[:],
                             start=True, stop=True)
            nc.tensor.matmul(wa_ps[:, 1:P + 1], WT_sb[:], a2b[:],
                             start=True, stop=True)
            nc.scalar.copy(wa1_sb[:], wa_ps[:, 0:1])
            nc.vector.tensor_copy(wa2b_sb[:], wa_ps[:, 1:P + 1])

            QB = 4 if n_t % 4 == 0 else 2
            for t2 in range(0, n_t, QB):
                hT_ps = prep_ps.tile([P, QB * P], F16, tag="hT_ps")
                for k in range(QB):
                    t = t2 + k
                    hc = h_chunks[(t // HCH) * HCH]
                    nc.tensor.transpose(hT_ps[:, k * P:k * P + F],
                                        hc[:, (t % HCH) * F:(t % HCH + 1) * F],
                                        ident16[:])
                hT_sb = prep.tile([F, QB * P], F16)
                nc.scalar.copy(hT_sb[:], hT_ps[:F, 0:QB * P])
                # s[p, t] = (H_t @ (W a1))[p]
                s_q = prep_ps1.tile([P, QB], F32, tag="s_q")
                for k in range(QB):
                    nc.tensor.matmul(s_q[:, k:k + 1],
                                     hT_sb[:, k * P:(k + 1) * P],
                                     wa1_sb[:], start=True, stop=True)
                nc.vector.tensor_copy(s_sb[:, t2:t2 + QB], s_q[:])
                # n_bcast[p, slice] = n[slice] broadcast over partitions
                nb_ps = prep_ps.tile([P, QB * P], F32, tag="nb_ps")
                nc.tensor.matmul(nb_ps[:], wa2b_sb[:], hT_sb[:],
                                 start=True, stop=True)
                nc.vector.tensor_copy(n_bcast[:, t2 * P:(t2 + QB) * P],
                                      nb_ps[:])
                # X tiles straight from h^T: X_t = H_t @ W
                for k in range(QB):
                    t = t2 + k
                    x_ps = prep_ps.tile([P, U], F32, tag="xq")
                    nc.tensor.matmul(x_ps[:], hT_sb[:, k * P:(k + 1) * P],
                                     W_sb[:], start=True, stop=True)
                    nc.vector.tensor_copy(
                        Xp_sb[:, t * (U + 1):t * (U + 1) + U], x_ps[:])
                # z_b = exp(0.2 n), en_b = exp(n) straight from PSUM on ACT
                nc.scalar.activation(z_b[:, t2 * P:(t2 + QB) * P], nb_ps[:],
                                     AF.Exp, scale=LEAKY_SLOPE)
                nc.scalar.activation(en_b[:, t2 * P:(t2 + QB) * P], nb_ps[:],
                                     AF.Exp)
            # w = exp(0.2 s), es = exp(s) per-partition scalars
            nc.scalar.activation(w_sb[:], s_sb[:], AF.Exp, scale=LEAKY_SLOPE)
            nc.scalar.activation(es_sb[:], s_sb[:], AF.Exp)

        # ---------------- main loop over query tiles ----------------
        p5set = set(i for i in P5_ITERS if i < n_t)
        GROUP = 16                     # transposes per PSUM tile (2 banks)
        n_groups = (n_t + GROUP - 1) // GROUP
        LOOK = 2                       # produce lookahead (iters)

        with tc.tile_pool(name="x1pool", bufs=2) as x1pool, \
             tc.tile_pool(name="x2pool", bufs=1) as x2pool, \
             tc.tile_pool(name="ppool", bufs=LOOK + 2) as ppool, \
             tc.tile_pool(name="ptpool", bufs=4) as ptpool, \
             tc.tile_pool(name="psT", bufs=3, space="PSUM") as psT, \
             tc.tile_pool(name="psAcc", bufs=2, space="PSUM") as psAcc:

            p_tiles = {}
            acc_tiles = {}

            def produce(it):
                load_a()               # keep the gpsimd queue fed, in order
                s_bias = s_sb[:, it:it + 1]
                p_t = ppool.tile([P, n_nodes], F16, tag="p")
                if it in p5set:
                    # pure-DVE: x1 = es_i*en_j, x2 = w_i*z_j, p = max
                    x1 = x1pool.tile([P, n_nodes], F16, tag="x1")
                    nc.vector.tensor_scalar_mul(x1[:], en_b[:],
                                                es_sb[:, it:it + 1])
                    x2 = x2pool.tile([P, n_nodes], F16, tag="x2")
                    nc.vector.tensor_scalar_mul(x2[:], z_b[:], w_sb[:, it:it + 1])
                    nc.vector.tensor_max(p_t[:], x1[:], x2[:])
                else:
                    # ACT-heavy: Prelu then Exp (both ScalarE, no DVE)
                    el = x1pool.tile([P, n_nodes], F16, tag="el")
                    nc.scalar.activation(el[:], n_bcast[:], AF.Prelu,
                                         bias=s_bias, scale=1.0,
                                         alpha=LEAKY_SLOPE)
                    nc.scalar.activation(p_t[:], el[:], AF.Exp)
                p_tiles[it] = p_t

            def consume(it):
                a_t = a_tiles.pop(it)
                p_t = p_tiles.pop(it)
                fine = it >= n_t - 4   # tail iterations: 8-block pipelining
                half = n_nodes // 2
                if not fine:
                    # mask in place on DVE, one full pass (fewer drains;
                    # never GpSimd: its tensor ops contend with DVE 2-port
                    # mode and slow everything down)
                    nc.vector.tensor_mul(p_t[:], p_t[:], a_t[:])

                # transpose P_m 128x128 blocks -> PSUM, copy groups to SBUF
                acc_ps = psAcc.tile([P, U + 1], F32, tag="acc_ps")
                for g in range(n_groups):
                    k_n = min(GROUP, n_t - g * GROUP)
                    pt_ps = psT.tile([P, GROUP * P], F16, tag="pt_ps")
                    for half_g in range(2 if fine else 1):
                        if fine:
                            lo = g * GROUP * P + half_g * (GROUP // 2) * P
                            hi = lo + (GROUP // 2) * P
                            nc.vector.tensor_mul(p_t[:, lo:hi], p_t[:, lo:hi],
                                                 a_t[:, lo:hi])
                            ks = range(half_g * (GROUP // 2),
                                       min(k_n, (half_g + 1) * (GROUP // 2)))
                        else:
                            ks = range(k_n)
                        for k in ks:
                            jt = g * GROUP + k
                            nc.tensor.transpose(pt_ps[:, k * P:(k + 1) * P],
                                                p_t[:, jt * P:(jt + 1) * P],
                                                ident16[:])
                    pt_sb = ptpool.tile([P, GROUP * P], F16, tag="pt_sb")
                    w_n = k_n * P
                    if fine:
                        # split the copy across both engines in the tail
                        nc.scalar.copy(pt_sb[:, 0:w_n // 2], pt_ps[:, 0:w_n // 2])
                        nc.vector.tensor_copy(pt_sb[:, w_n // 2:w_n],
                                              pt_ps[:, w_n // 2:w_n])
                    else:
                        nc.vector.tensor_copy(pt_sb[:, 0:w_n], pt_ps[:, 0:w_n])
                    # H_cap accumulation for this group's j tiles
                    for k in range(k_n):
                        jt = g * GROUP + k
                        nc.tensor.matmul(
                            acc_ps[:], pt_sb[:, k * P:(k + 1) * P],
                            Xp_sb[:, jt * (U + 1):(jt + 1) * (U + 1)],
                            start=(jt == 0), stop=(jt == n_t - 1))

                # pair up reciprocals (fewer DVE ops/drains); emit(it-1)
                # runs after consume(it) in the same loop body, so dinv for
                # the pair is ready exactly when needed
                if fine:
                    # immediate emit needs dinv now
                    nc.vector.reciprocal(dinv_sb[:, it:it + 1],
                                         acc_ps[:, U:U + 1])
                else:
                    nc.vector.tensor_copy(rs_sb[:, it % 2:it % 2 + 1],
                                          acc_ps[:, U:U + 1])
                    if it % 2 == 1:
                        nc.vector.reciprocal(dinv_sb[:, it - 1:it + 1],
                                             rs_sb[:, 0:2])
                acc_tiles[it] = acc_ps

            def emit_out(it):
                # out = relu(H_cap[:, :U] / H_cap[:, U]) -- relu+scale on ACT,
                # into the SBUF output buffer (flushed by DMA at the end).
                acc_ps = acc_tiles.pop(it)
                nc.scalar.activation(outsbuf[:, it * U:(it + 1) * U],
                                     acc_ps[:, 0:U], AF.Relu,
                                     scale=dinv_sb[:, it:it + 1])

            for it in range(n_t + LOOK + 1):
                if it < n_t:
                    produce(it)
                if LOOK <= it < n_t + LOOK:
                    ct = it - LOOK
                    consume(ct)
                    if ct >= n_t - 4:
                        emit_out(ct)
                if LOOK < it < n_t + LOOK - 3:
                    emit_out(it - LOOK - 1)
                if FLUSH_SPLIT and it == n_t * 5 // 8:
                    hn = n_t // 2
                    nc.sync.dma_start(
                        out_d[0:hn * P, :].rearrange("(t p) u -> p t u", p=P),
                        outsbuf[:, 0:hn * U].rearrange("p (t u) -> p t u", u=U))

            # final output flush on the idle sync ring
            lo = (n_t // 2) if FLUSH_SPLIT else 0
            nc.sync.dma_start(
                out_d[lo * P:n_t * P, :].rearrange("(t p) u -> p t u", p=P),
                outsbuf[:, lo * U:n_t * U].rearrange("p (t u) -> p t u", u=U))

    nc.compile()
    return nc


_NC_CACHE = {}


def _get_nc(n_nodes=N_NODES):
    if n_nodes not in _NC_CACHE:
        _NC_CACHE[n_nodes] = build_nc(n_nodes)
    return _NC_CACHE[n_nodes]


def kernel(H, A, W, a_1, a_2):
    """Full inputs in, full output out. Shards batch across 8 NeuronCores."""
    import os
    # The axon trace path needs antenv.axon_hooks, which this image lacks;
    # make sure an inherited BASS_TRACE can't route us there.
    os.environ["BASS_NEVER_TRACE"] = "1"
    from concourse.bass_utils import run_bass_kernel_spmd

    B = H.shape[0]
    assert B == N_CORES
    nc = _get_nc(H.shape[1])
    in_maps = [
        {
            "H": np.ascontiguousarray(H[b], dtype=np.float32),
            "A": np.ascontiguousarray(A[b], dtype=np.float32),
            "W": np.ascontiguousarray(W, dtype=np.float32),
            "a_1": np.ascontiguousarray(a_1, dtype=np.float32),
            "a_2": np.ascontiguousarray(a_2, dtype=np.float32),
        }
        for b in range(B)
    ]
    res = run_bass_kernel_spmd(nc, in_maps, core_ids=list(range(N_CORES)))
    out = np.stack([res.results[b]["out"] for b in range(B)]).astype(np.float32)
    return out
